# revision 6
# baseline (speedup 1.0000x reference)
import sys

sys.path.insert(0, "/opt/trn_rl_repo")

import numpy as np

B, L, D = 256, 2048, 128
N_CORES = 8
BPC = B // N_CORES  # 32 batch rows per core
P = 128

_compiled = None


def _host_consts():
    # u is a compile-time constant in the reference (key(42)); its stable
    # argsort s and inverse sinv are precomputed on host.
    import jax

    with jax.default_device(jax.devices("cpu")[0]):
        u = np.asarray(
            jax.random.uniform(jax.random.key(42), (B, L), dtype=jax.numpy.float32)
        )
    s = np.argsort(u, axis=1, kind="stable").astype(np.int32)  # [B, L]
    sinv = np.empty_like(s)
    rows = np.arange(B)[:, None]
    sinv[rows, s] = np.arange(L)[None, :]
    return s, sinv


def _build_program():
    import concourse.bass as bass
    import concourse.tile as tile
    from concourse import bacc, mybir

    f32 = mybir.dt.float32
    i32 = mybir.dt.int32
    i16 = mybir.dt.int16

    nc = bacc.Bacc("TRN2", target_bir_lowering=False, debug=False, num_devices=1)

    d_x = nc.dram_tensor("d_x", [BPC * L, D], f32, kind="ExternalInput")
    d_mask = nc.dram_tensor("d_mask", [BPC, L], i32, kind="ExternalInput")
    d_s16 = nc.dram_tensor("d_s16", [P, L], i16, kind="ExternalInput")
    d_svlo = nc.dram_tensor("d_svlo", [P, L], i16, kind="ExternalInput")
    d_svhi = nc.dram_tensor("d_svhi", [P, L], i16, kind="ExternalInput")
    o_xp = nc.dram_tensor("o_xp", [BPC, L, D], f32, kind="ExternalOutput")
    o_perm = nc.dram_tensor("o_perm", [BPC, L], i32, kind="ExternalOutput")

    H = L // 2  # 1024, local_scatter window width

    with tile.TileContext(nc) as tc:
        with tc.tile_pool(name="main", bufs=1) as pool:
            t_s16 = pool.tile([P, L], i16)
            t_svlo = pool.tile([P, L], i16)
            t_svhi = pool.tile([P, L], i16)
            nc.sync.dma_start(t_s16[:, :], d_s16[:, :])
            nc.sync.dma_start(t_svlo[:, :], d_svlo[:, :])
            nc.sync.dma_start(t_svhi[:, :], d_svhi[:, :])

            t_iota0 = pool.tile([P, L], i16)
            t_iota1 = pool.tile([P, L], i16)
            nc.gpsimd.iota(t_iota0[:, :], [[1, L]], base=0, channel_multiplier=0)
            nc.gpsimd.iota(t_iota1[:, :], [[1, L]], base=1, channel_multiplier=0)

            # ---- mask load: rows at partitions 4b (s-domain) and 4b+1 (natural) ----
            t_m32 = pool.tile([P, L], i32)
            t_mask = pool.tile([P, L], i16)
            nc.vector.memset(t_m32[:, :], 0)
            nc.sync.dma_start(t_m32[0 : 4 * BPC : 4, :], d_mask[:, :])
            nc.vector.tensor_copy(t_mask[:, :], t_m32[:, :])
            nc.sync.dma_start(t_mask[1 : 4 * BPC : 4, :], t_mask[0 : 4 * BPC : 4, :])

            # ---- ms scatter: T[4b, sinv[j]] = mask[j]; natural mask at 4b+1 ----
            t_T = pool.tile([P, L], i16)
            nc.gpsimd.local_scatter(
                t_T[:, 0:H], t_mask[:, :], t_svlo[:, :], channels=P, num_elems=H, num_idxs=L
            )
            nc.gpsimd.local_scatter(
                t_T[:, H:L], t_mask[:, :], t_svhi[:, :], channels=P, num_elems=H, num_idxs=L
            )
            # strided DMA sources need start offset 0; rows 0::4 hold the same
            # natural mask values as 1::4
            nc.sync.dma_start(t_T[1 : 4 * BPC : 4, :], t_mask[0 : 4 * BPC : 4, :])

            # ---- inclusive prefix sum along free dim (11 rounds, ping-pong) ----
            t_U = pool.tile([P, L], i16)
            src, dst = t_T, t_U
            sh = 1
            while sh < L:
                nc.vector.tensor_copy(dst[:, 0:sh], src[:, 0:sh])
                nc.vector.tensor_add(dst[:, sh:L], src[:, sh:L], src[:, 0 : L - sh])
                src, dst = dst, src
                sh *= 2
            t_PP = src  # inclusive prefix: P_s at 4b, Pm at 4b+1

            # ---- exclusive prefix and element value ----
            t_EX = pool.tile([P, L], i16)
            nc.vector.memset(t_EX[:, 0:1], 0)
            nc.vector.tensor_copy(t_EX[:, 1:L], t_PP[:, 0 : L - 1])
            t_MS = pool.tile([P, L], i16)
            nc.vector.tensor_sub(t_MS[:, :], t_PP[:, :], t_EX[:, :])

            # ---- treal = (EX + 1) * MS - 1  (valid at 4b) ----
            t_tr = pool.tile([P, L], i16)
            nc.vector.scalar_tensor_tensor(
                t_tr[:, :], t_EX[:, :], 1, t_MS[:, :],
                op0=mybir.AluOpType.add, op1=mybir.AluOpType.mult,
            )
            nc.vector.tensor_scalar_sub(t_tr[:, :], t_tr[:, :], 1)

            # ---- tpad = (R + iota1 - EX) * (1 - MS) - 1  (valid at 4b+1) ----
            t_a = pool.tile([P, L], i16)
            nc.vector.tensor_sub(t_a[:, :], t_iota1[:, :], t_EX[:, :])
            nc.vector.tensor_add(
                t_a[:, :], t_a[:, :], t_PP[:, L - 1 : L].to_broadcast([P, L])
            )
            t_om = pool.tile([P, L], i16)
            nc.vector.tensor_scalar(
                t_om[:, :], t_MS[:, :], -1, 1,
                op0=mybir.AluOpType.mult, op1=mybir.AluOpType.add,
            )
            t_tp = pool.tile([P, L], i16)
            nc.vector.tensor_mul(t_tp[:, :], t_a[:, :], t_om[:, :])
            nc.vector.tensor_scalar_sub(t_tp[:, :], t_tp[:, :], 1)

            # ---- window splits: lo = t - 4096*(t>=H); hi = t - H ----
            def win_split(t_in, t_lo, t_hi):
                nc.vector.tensor_scalar(
                    t_lo[:, :], t_in[:, :], H, -4096,
                    op0=mybir.AluOpType.is_ge, op1=mybir.AluOpType.mult,
                )
                nc.vector.tensor_add(t_lo[:, :], t_lo[:, :], t_in[:, :])
                nc.vector.tensor_scalar_sub(t_hi[:, :], t_in[:, :], H)

            t_trlo = pool.tile([P, L], i16)
            t_trhi = pool.tile([P, L], i16)
            win_split(t_tr, t_trlo, t_trhi)
            t_tplo = pool.tile([P, L], i16)
            t_tphi = pool.tile([P, L], i16)
            win_split(t_tp, t_tplo, t_tphi)

            # ---- scatter perm streams ----
            t_pa = pool.tile([P, L], i16)  # real stream, valid at 4b
            nc.gpsimd.local_scatter(
                t_pa[:, 0:H], t_s16[:, :], t_trlo[:, :], channels=P, num_elems=H, num_idxs=L
            )
            nc.gpsimd.local_scatter(
                t_pa[:, H:L], t_s16[:, :], t_trhi[:, :], channels=P, num_elems=H, num_idxs=L
            )
            t_pb = pool.tile([P, L], i16)  # pad stream, valid at 4b+1
            nc.gpsimd.local_scatter(
                t_pb[:, 0:H], t_iota0[:, :], t_tplo[:, :], channels=P, num_elems=H, num_idxs=L
            )
            nc.gpsimd.local_scatter(
                t_pb[:, H:L], t_iota0[:, :], t_tphi[:, :], channels=P, num_elems=H, num_idxs=L
            )

            # ---- combine: perm16 = pa + shift(pb 4b+1 -> 4b) ----
            # full shift-by-one instead of strided-offset source (broken);
            # only rows 0::4 of t_pbs are ever read
            t_pbs = pool.tile([P, L], i16)
            nc.vector.memset(t_pbs[:, :], 0)
            nc.sync.dma_start(t_pbs[0:127, :], t_pb[1:128, :])
            t_p16 = pool.tile([P, L], i16)
            nc.vector.tensor_add(t_p16[:, :], t_pa[:, :], t_pbs[:, :])

            t_p32 = pool.tile([P, L], i32)
            nc.vector.tensor_copy(t_p32[:, :], t_p16[:, :])
            nc.sync.dma_start(o_perm[:, :], t_p32[0 : 4 * BPC : 4, :])

            # ---- build gather idx: idx[16g+l, 128b+k] = perm_b[16k+l] ----
            t_W = pool.tile([P, 32 * BPC], i16)
            nc.vector.memset(t_W[:, :], 0)
            for b in range(BPC):
                nc.sync.dma_start(
                    t_W[:, 32 * b : 32 * b + 16], t_p16[4 * b : 4 * b + 1, :]
                )
            t_Tr = pool.tile([P, 32 * BPC], i16)
            nc.vector.transpose(t_Tr[:, :], t_W[:, :])
            t_idx = pool.tile([P, 128 * BPC], i16)
            for a in range(4):
                nc.sync.dma_start(
                    t_idx[0:16, :].rearrange("l (b m) -> l b m", m=128)[
                        :, :, 32 * a : 32 * a + 32
                    ],
                    t_Tr[32 * a : 32 * a + 16, :].rearrange("l (b m) -> l b m", m=32),
                )
            nc.sync.dma_start(t_idx[16:32, :], t_idx[0:16, :])
            nc.sync.dma_start(t_idx[32:64, :], t_idx[0:32, :])
            nc.sync.dma_start(t_idx[64:128, :], t_idx[0:64, :])

        # ---- phase B: per-batch gather + store (pipelined pool) ----
        with tc.tile_pool(name="xmov", bufs=3) as xpool:
            for b in range(BPC):
                t_xg = xpool.tile([P, 16, D], f32)
                # HW caps dma_gather at 1024 idxs/call: split into 2 halves
                for h in range(2):
                    nc.gpsimd.dma_gather(
                        t_xg[:, 8 * h : 8 * (h + 1), :],
                        d_x[b * L : (b + 1) * L, :],
                        t_idx[:, 128 * b + 64 * h : 128 * b + 64 * (h + 1)],
                        num_idxs=1024,
                        num_idxs_reg=1024,
                        elem_size=D,
                    )
                nc.sync.dma_start(
                    o_xp[b, :, :].rearrange("(c p) d -> p c d", p=128),
                    t_xg[:, :, :],
                )

    nc.compile()
    return nc


def _get_compiled():
    global _compiled
    if _compiled is None:
        s, sinv = _host_consts()
        nc = _build_program()
        _compiled = (nc, s, sinv)
    return _compiled


def _make_in_maps(x, mask, s, sinv):
    x = np.ascontiguousarray(np.asarray(x, dtype=np.float32))
    mask = np.ascontiguousarray(np.asarray(mask, dtype=np.int32))
    in_maps = []
    for c in range(N_CORES):
        b0 = c * BPC
        s16 = np.zeros((P, L), dtype=np.int16)
        svlo = np.full((P, L), -1, dtype=np.int16)
        svhi = np.full((P, L), -1, dtype=np.int16)
        for b in range(BPC):
            sb = s[b0 + b]
            vb = sinv[b0 + b]
            s16[4 * b, :] = sb.astype(np.int16)
            svlo[4 * b, :] = np.where(vb < 1024, vb, -1).astype(np.int16)
            svhi[4 * b, :] = np.where(vb >= 1024, vb - 1024, -1).astype(np.int16)
        in_maps.append(
            {
                "d_x": x[b0 : b0 + BPC].reshape(BPC * L, D),
                "d_mask": mask[b0 : b0 + BPC],
                "d_s16": s16,
                "d_svlo": svlo,
                "d_svhi": svhi,
            }
        )
    return in_maps


def kernel(x: np.ndarray, mask: np.ndarray):
    from concourse.bass_utils import run_bass_kernel_spmd

    nc, s, sinv = _get_compiled()
    in_maps = _make_in_maps(x, mask, s, sinv)
    res = run_bass_kernel_spmd(nc, in_maps, list(range(N_CORES)))
    xp = np.empty((B, L, D), dtype=np.float32)
    perm = np.empty((B, L), dtype=np.int32)
    for c in range(N_CORES):
        b0 = c * BPC
        xp[b0 : b0 + BPC] = np.asarray(res.results[c]["o_xp"])
        perm[b0 : b0 + BPC] = np.asarray(res.results[c]["o_perm"])
    return xp, perm


# revision 16
# speedup vs baseline: 1.4110x; 1.4110x over previous
import sys

sys.path.insert(0, "/opt/trn_rl_repo")

import numpy as np

B, L, D = 256, 2048, 128
N_CORES = 8
BPC = B // N_CORES  # 32 batch rows per core
P = 128

_compiled = None


def _host_consts():
    # u is a compile-time constant in the reference (key(42)); its stable
    # argsort s and inverse sinv are precomputed on host.
    import jax

    with jax.default_device(jax.devices("cpu")[0]):
        u = np.asarray(
            jax.random.uniform(jax.random.key(42), (B, L), dtype=jax.numpy.float32)
        )
    s = np.argsort(u, axis=1, kind="stable").astype(np.int32)  # [B, L]
    sinv = np.empty_like(s)
    rows = np.arange(B)[:, None]
    sinv[rows, s] = np.arange(L)[None, :]
    return s, sinv


def _build_program():
    import concourse.bass as bass
    import concourse.tile as tile
    from concourse import bacc, mybir

    f32 = mybir.dt.float32
    i32 = mybir.dt.int32
    i16 = mybir.dt.int16

    nc = bacc.Bacc("TRN2", target_bir_lowering=False, debug=False, num_devices=1)

    d_x = nc.dram_tensor("d_x", [BPC * L, D], f32, kind="ExternalInput")
    d_mask = nc.dram_tensor("d_mask", [BPC, L], i32, kind="ExternalInput")
    d_s16 = nc.dram_tensor("d_s16", [P, L], i16, kind="ExternalInput")
    d_svlo = nc.dram_tensor("d_svlo", [P, L], i16, kind="ExternalInput")
    d_svhi = nc.dram_tensor("d_svhi", [P, L], i16, kind="ExternalInput")
    o_xp = nc.dram_tensor("o_xp", [BPC, L, D], f32, kind="ExternalOutput")
    o_perm = nc.dram_tensor("o_perm", [BPC, L], i32, kind="ExternalOutput")

    H = L // 2  # 1024, local_scatter window width

    with tile.TileContext(nc) as tc:
        # spread large DMAs across otherwise-idle engine queues
        engines = [nc.sync, nc.scalar]
        with tc.tile_pool(name="main", bufs=1) as pool:
            t_s16 = pool.tile([P, L], i16)
            t_svlo = pool.tile([P, L], i16)
            t_svhi = pool.tile([P, L], i16)
            nc.scalar.dma_start(t_svlo[:, :], d_svlo[:, :])
            nc.sync.dma_start(t_svhi[:, :], d_svhi[:, :])
            nc.scalar.dma_start(t_s16[:, :], d_s16[:, :])

            # CoreSim requires fully-initialized reads; memsets run while DVE
            # is idle waiting on the first loads (off the critical path)
            t_m32 = pool.tile([P, L], i32)
            t_pbs = pool.tile([P, L], i16)
            t_W = pool.tile([P, 32 * BPC], i16)
            nc.vector.memset(t_m32[:, :], 0)
            nc.vector.memset(t_pbs[:, :], 0)
            nc.vector.memset(t_W[:, :], 0)

            t_iota0 = pool.tile([P, L], i16)
            t_iota1 = pool.tile([P, L], i16)
            nc.gpsimd.iota(t_iota0[:, :], [[1, L]], base=0, channel_multiplier=0)
            nc.gpsimd.iota(t_iota1[:, :], [[1, L]], base=1, channel_multiplier=0)

            # ---- mask load: rows at partitions 4b (s-domain) and 4b+1 (natural) ----
            t_mask = pool.tile([P, L], i16)
            nc.sync.dma_start(t_m32[0 : 4 * BPC : 4, :], d_mask[:, :])
            nc.vector.tensor_copy(t_mask[:, :], t_m32[:, :])
            nc.sync.dma_start(t_mask[1 : 4 * BPC : 4, :], t_mask[0 : 4 * BPC : 4, :])

            # ---- ms scatter: T[4b, sinv[j]] = mask[j]; natural mask at 4b+1 ----
            t_T = pool.tile([P, L], i16)
            nc.gpsimd.local_scatter(
                t_T[:, 0:H], t_mask[:, :], t_svlo[:, :], channels=P, num_elems=H, num_idxs=L
            )
            nc.gpsimd.local_scatter(
                t_T[:, H:L], t_mask[:, :], t_svhi[:, :], channels=P, num_elems=H, num_idxs=L
            )
            # strided DMA sources need start offset 0; rows 0::4 hold the same
            # natural mask values as 1::4
            nc.sync.dma_start(t_T[1 : 4 * BPC : 4, :], t_mask[0 : 4 * BPC : 4, :])

            # ---- inclusive prefix sum along free dim (11 rounds, ping-pong) ----
            t_U = pool.tile([P, L], i16)
            src, dst = t_T, t_U
            sh = 1
            while sh < L:
                nc.vector.tensor_copy(dst[:, 0:sh], src[:, 0:sh])
                nc.vector.tensor_add(dst[:, sh:L], src[:, sh:L], src[:, 0 : L - sh])
                src, dst = dst, src
                sh *= 2
            t_PP = src  # inclusive prefix: P_s at 4b, Pm at 4b+1

            # ---- exclusive prefix and element value ----
            t_EX = pool.tile([P, L], i16)
            nc.vector.memset(t_EX[:, 0:1], 0)
            nc.vector.tensor_copy(t_EX[:, 1:L], t_PP[:, 0 : L - 1])
            t_MS = pool.tile([P, L], i16)
            nc.vector.tensor_sub(t_MS[:, :], t_PP[:, :], t_EX[:, :])

            # ---- treal = (EX + 1) * MS - 1  (valid at 4b) ----
            t_tr = pool.tile([P, L], i16)
            nc.vector.scalar_tensor_tensor(
                t_tr[:, :], t_EX[:, :], 1, t_MS[:, :],
                op0=mybir.AluOpType.add, op1=mybir.AluOpType.mult,
            )
            nc.vector.tensor_scalar_sub(t_tr[:, :], t_tr[:, :], 1)

            # ---- tpad = (R + iota1 - EX) * (1 - MS) - 1  (valid at 4b+1) ----
            t_a = pool.tile([P, L], i16)
            nc.vector.tensor_sub(t_a[:, :], t_iota1[:, :], t_EX[:, :])
            nc.vector.tensor_add(
                t_a[:, :], t_a[:, :], t_PP[:, L - 1 : L].to_broadcast([P, L])
            )
            t_om = pool.tile([P, L], i16)
            nc.vector.tensor_scalar(
                t_om[:, :], t_MS[:, :], -1, 1,
                op0=mybir.AluOpType.mult, op1=mybir.AluOpType.add,
            )
            t_tp = pool.tile([P, L], i16)
            nc.vector.tensor_mul(t_tp[:, :], t_a[:, :], t_om[:, :])
            nc.vector.tensor_scalar_sub(t_tp[:, :], t_tp[:, :], 1)

            # ---- window splits: lo = t - 4096*(t>=H); hi = t - H ----
            def win_split(t_in, t_lo, t_hi):
                nc.vector.tensor_scalar(
                    t_lo[:, :], t_in[:, :], H, -4096,
                    op0=mybir.AluOpType.is_ge, op1=mybir.AluOpType.mult,
                )
                nc.vector.tensor_add(t_lo[:, :], t_lo[:, :], t_in[:, :])
                nc.vector.tensor_scalar_sub(t_hi[:, :], t_in[:, :], H)

            t_trlo = pool.tile([P, L], i16)
            t_trhi = pool.tile([P, L], i16)
            win_split(t_tr, t_trlo, t_trhi)
            t_tplo = pool.tile([P, L], i16)
            t_tphi = pool.tile([P, L], i16)
            win_split(t_tp, t_tplo, t_tphi)

            # ---- scatter perm streams ----
            t_pa = pool.tile([P, L], i16)  # real stream, valid at 4b
            nc.gpsimd.local_scatter(
                t_pa[:, 0:H], t_s16[:, :], t_trlo[:, :], channels=P, num_elems=H, num_idxs=L
            )
            nc.gpsimd.local_scatter(
                t_pa[:, H:L], t_s16[:, :], t_trhi[:, :], channels=P, num_elems=H, num_idxs=L
            )
            t_pb = pool.tile([P, L], i16)  # pad stream, valid at 4b+1
            nc.gpsimd.local_scatter(
                t_pb[:, 0:H], t_iota0[:, :], t_tplo[:, :], channels=P, num_elems=H, num_idxs=L
            )
            nc.gpsimd.local_scatter(
                t_pb[:, H:L], t_iota0[:, :], t_tphi[:, :], channels=P, num_elems=H, num_idxs=L
            )

            # ---- combine: perm16 = pa + shift(pb 4b+1 -> 4b) ----
            # full shift-by-one instead of strided-offset source (broken);
            # only rows 0::4 of t_pbs are ever read
            nc.sync.dma_start(t_pbs[0:127, :], t_pb[1:128, :])
            t_p16 = pool.tile([P, L], i16)
            nc.vector.tensor_add(t_p16[:, :], t_pa[:, :], t_pbs[:, :])

            # ---- build gather idx: idx[16g+l, 128b+k] = perm_b[16k+l] ----
            for b in range(BPC):
                engines[b % 2].dma_start(
                    t_W[:, 32 * b : 32 * b + 16], t_p16[4 * b : 4 * b + 1, :]
                )
            t_Tr = pool.tile([P, 32 * BPC], i16)
            nc.vector.transpose(t_Tr[:, :], t_W[:, :])
            t_idx = pool.tile([P, 128 * BPC], i16)
            for a in range(4):
                engines[a % 2].dma_start(
                    t_idx[0:16, :].rearrange("l (b m) -> l b m", m=128)[
                        :, :, 32 * a : 32 * a + 32
                    ],
                    t_Tr[32 * a : 32 * a + 16, :].rearrange("l (b m) -> l b m", m=32),
                )
            nc.sync.dma_start(t_idx[16:32, :], t_idx[0:16, :])
            nc.scalar.dma_start(t_idx[32:64, :], t_idx[0:32, :])
            nc.sync.dma_start(t_idx[64:128, :], t_idx[0:64, :])

            # o_perm store is off the critical path to phase B: issue it last
            t_p32 = pool.tile([P, L], i32)
            nc.vector.tensor_copy(t_p32[:, :], t_p16[:, :])
            nc.sync.dma_start(o_perm[:, :], t_p32[0 : 4 * BPC : 4, :])

        # ---- phase B: per-batch gather + store (pipelined pool) ----
        with tc.tile_pool(name="xmov", bufs=3) as xpool:
            for b in range(BPC):
                t_xg = xpool.tile([P, 16, D], f32)
                # HW caps dma_gather at 1024 idxs/call: split into 2 halves
                for h in range(2):
                    nc.gpsimd.dma_gather(
                        t_xg[:, 8 * h : 8 * (h + 1), :],
                        d_x[b * L : (b + 1) * L, :],
                        t_idx[:, 128 * b + 64 * h : 128 * b + 64 * (h + 1)],
                        num_idxs=1024,
                        num_idxs_reg=1024,
                        elem_size=D,
                    )
                # t_xg[16a+l, c] = x[perm_b[128l + 8c + a]] under the new idx layout
                engines[b % 2].dma_start(
                    o_xp[b, :, :].rearrange("(c p) d -> p c d", p=128),
                    t_xg[:, :, :],
                )

    nc.compile()
    return nc


def _get_compiled():
    global _compiled
    if _compiled is None:
        s, sinv = _host_consts()
        nc = _build_program()
        _compiled = (nc, s, sinv)
    return _compiled


def _make_in_maps(x, mask, s, sinv):
    x = np.ascontiguousarray(np.asarray(x, dtype=np.float32))
    mask = np.ascontiguousarray(np.asarray(mask, dtype=np.int32))
    in_maps = []
    for c in range(N_CORES):
        b0 = c * BPC
        s16 = np.zeros((P, L), dtype=np.int16)
        svlo = np.full((P, L), -1, dtype=np.int16)
        svhi = np.full((P, L), -1, dtype=np.int16)
        for b in range(BPC):
            sb = s[b0 + b]
            vb = sinv[b0 + b]
            s16[4 * b, :] = sb.astype(np.int16)
            svlo[4 * b, :] = np.where(vb < 1024, vb, -1).astype(np.int16)
            svhi[4 * b, :] = np.where(vb >= 1024, vb - 1024, -1).astype(np.int16)
        in_maps.append(
            {
                "d_x": x[b0 : b0 + BPC].reshape(BPC * L, D),
                "d_mask": mask[b0 : b0 + BPC],
                "d_s16": s16,
                "d_svlo": svlo,
                "d_svhi": svhi,
            }
        )
    return in_maps


def kernel(x: np.ndarray, mask: np.ndarray):
    from concourse.bass_utils import run_bass_kernel_spmd

    nc, s, sinv = _get_compiled()
    in_maps = _make_in_maps(x, mask, s, sinv)
    res = run_bass_kernel_spmd(nc, in_maps, list(range(N_CORES)))
    xp = np.empty((B, L, D), dtype=np.float32)
    perm = np.empty((B, L), dtype=np.int32)
    for c in range(N_CORES):
        b0 = c * BPC
        xp[b0 : b0 + BPC] = np.asarray(res.results[c]["o_xp"])
        perm[b0 : b0 + BPC] = np.asarray(res.results[c]["o_perm"])
    return xp, perm
